# revision 2
# baseline (speedup 1.0000x reference)
"""CrystalEncoder Trainium2 kernel (v2).

Strategy: pure data parallel — one crystal (batch element) per NeuronCore.
The O(N) geometry (cart coords, pairwise d^2 / d feature rows, flattening)
is done on HOST; the O(N^2 * H) work (RBF expansion, gated message passing)
runs on-device in one fused Bass/Tile kernel.

Device dataflow per core (N=256 atoms, H=128, BINS=40, NL=2):
  1. rf64 [64, 2048] f32 input: 16 "fills" x 4 rows (d^2/d x 2 i-groups),
     each fill covering 8 i-rows x 256 j pairs per group. One cheap DMA
     (cost scales with per-partition bytes, so 64-partition packing wins).
  2. RBF exponents via K=64 matmuls: cE64 [64, 16*128] holds 16 per-fill
     selector blocks; exponent e = -g*d^2 + 2*g*c_k*d per (bin, group),
     bias -g*c_k^2 folded into the Exp activation; rbfT [128, 32768] bf16
     resident (groups at partitions 0/64).
  3. Per layer l: gate matmul (K=40 bf16, ewR stationary per group);
     softplus as Exp then Ln(x+1) in the single natural_log_exp table set
     (Ln over 8192-wide chunks); DVE/Pool multiply by broadcast h_j and
     2-step reduce over j -> aggT; node update zT = node_w^T @ aggT,
     Silu, residual + mask.
  4. Pooling: reduce over atoms -> sum_h [H, 1] -> DRAM.
Host: g = sum_h / (n_valid + 1e-6); mu / log_var projections.

Sync discipline: this walrus build supports at most ONE semaphore wait per
instruction; _install_wait_splitter() splits multi-wait instructions with
same-engine NoOp carriers (as the baseline did).
"""

import numpy as np
import ml_dtypes

B, N, H, LAT, NL, BINS = 8, 256, 128, 64, 2, 40
VMAX = 8.0
GAMMA = 1.0 / (VMAX / BINS) ** 2  # 25.0

G = 2                  # i-groups; bins at partition offsets 0 / 64
IPG = N // G           # 128 i-rows per group
LOCF = IPG * N         # 32768 pairs per group (free size of rbfT)
NFILL = 16             # rf fills
FILLP = 2048           # pairs per fill per group
IPF = FILLP // N       # 8 i-rows per fill per group
ECH = 2048             # pairs per Exp chunk in rbf stage (= FILLP)
BLK = 8192             # pairs per gate block (per group); 4 blocks per group
IPB = BLK // N         # 32 i-rows per block
MMF = 512              # matmul free size (one PSUM bank of f32)

# mul/add placement: 'pool' = tensor ops on gpsimd engine, 'dve' = on vector
MUL_ENGINE = "pool"

_CACHE = {}


def _install_wait_splitter():
    """This walrus build supports at most ONE semaphore wait per ISA
    instruction. Split every multi-wait instruction by inserting same-engine
    NoOp carriers, each holding one of the waits, immediately before it."""
    import bass_rust
    import concourse.tile as tile
    from concourse import mybir

    if getattr(tile.TileContext, "_wait_split_installed", False):
        return
    orig = tile.TileContext._lower_ordered_insts
    counter = [0]

    def patched(self, ordered):
        for insts in ordered.values():
            newl = []
            for inst in insts:
                si = inst.sync_info
                ow = list(si.on_wait) if (si is not None and si.on_wait) else []
                if len(ow) > 1 and inst.engine != mybir.EngineType.Unassigned:
                    for w in ow[:-1]:
                        counter[0] += 1
                        nop = bass_rust.InstNoOp(
                            name=f"wsplit_{counter[0]}", ins=[], outs=[]
                        )
                        nop.engine = inst.engine
                        nop.sync_info = bass_rust.SyncInfo(
                            on_wait=[w], on_update=[]
                        )
                        newl.append(nop)
                    inst.sync_info = bass_rust.SyncInfo(
                        on_wait=[ow[-1]], on_update=list(si.on_update or [])
                    )
                newl.append(inst)
            insts[:] = newl
        return orig(self, ordered)

    tile.TileContext._lower_ordered_insts = patched

    def patched_dab(self, tick_clock, wait_clock):
        from concourse.vector_clock import ScopedClock

        probe = self.nc.sync.nop()
        wait_clock.add_sem_waits(
            probe.ins, ScopedClock({None: tick_clock.global_clock})
        )
        si = probe.ins.sync_info
        ow = list(si.on_wait) if (si is not None and si.on_wait) else []
        if len(ow) > 1:
            probe.ins.sync_info = bass_rust.SyncInfo(
                on_wait=[ow[0]], on_update=list(si.on_update or [])
            )
            for w in ow[1:]:
                n2 = self.nc.sync.nop()
                n2.ins.sync_info = bass_rust.SyncInfo(on_wait=[w], on_update=[])
        self.nc.sync.drain()
        self.nc.all_engine_barrier()
        popped = self.nc._tile_sem_poison_stack.pop()
        assert popped is self._sem_poison
        self.nc.clear_and_free_semaphores(list(self.sems.allocated().values()))
        self.nc.all_engine_barrier()

    tile.TileContext._drain_and_barrier = patched_dab
    tile.TileContext._wait_split_installed = True


def _build_nc(reps=1):
    import concourse.bass as bass
    import concourse.tile as tile
    from concourse import mybir

    _install_wait_splitter()

    F32 = mybir.dt.float32
    BF16 = mybir.dt.bfloat16
    AF = mybir.ActivationFunctionType
    X = mybir.AxisListType
    ALU = mybir.AluOpType
    POOL = mybir.EngineType.Pool

    nc = bass.Bass("TRN2", target_bir_lowering=False, debug=False)

    def dep_nop(engine, aps):
        """Engine-local nop reading `aps`: pulls their producers' ticks into
        the engine's observed clock so later real instructions need at most
        one new semaphore wait."""
        nop = engine.nop(hint="dep").ins
        nop.ins = [engine.lower_ap(ap) for ap in aps]
        return nop

    d_rf = nc.dram_tensor("rf64", [64, NFILL * FILLP // 16], F32,
                          kind="ExternalInput")  # [64, 2048]
    d_cE = nc.dram_tensor("cE64", [64, NFILL * H], F32, kind="ExternalInput")
    d_cbias = nc.dram_tensor("cbias", [H, 1], F32, kind="ExternalInput")
    d_ewR = nc.dram_tensor("ewR", [H, NL * H], BF16, kind="ExternalInput")
    d_ebT = nc.dram_tensor("ebT", [H, NL], F32, kind="ExternalInput")
    d_nwT = nc.dram_tensor("nwT", [H, NL * H], F32, kind="ExternalInput")
    d_nbT = nc.dram_tensor("nbT", [H, NL], F32, kind="ExternalInput")
    d_h0T = nc.dram_tensor("h0T", [H, N], F32, kind="ExternalInput")
    d_maskF = nc.dram_tensor("maskF", [H, N], F32, kind="ExternalInput")
    d_sumh = nc.dram_tensor("sumh", [H, 1], F32, kind="ExternalOutput")

    mul_eng = nc.gpsimd if MUL_ENGINE == "pool" else nc.vector

    with tile.TileContext(nc) as tc:
        with tc.tile_pool(name="consts", bufs=1) as consts:
            kw = dict(forced_dma_engine=POOL)
            t_rf = consts.tile_from(d_rf[:], **kw)
            t_cE = consts.tile_from(d_cE[:], **kw)
            t_cbias = consts.tile_from(d_cbias[:], **kw)
            t_ewR = consts.tile_from(d_ewR[:], **kw)
            t_ebT = consts.tile_from(d_ebT[:], **kw)
            t_nwT = consts.tile_from(d_nwT[:], **kw)
            t_nbT = consts.tile_from(d_nbT[:], **kw)
            t_hT = consts.tile_from(d_h0T[:], **kw)
            t_maskF = consts.tile_from(d_maskF[:], **kw)

            rbfT = consts.tile([H, LOCF], BF16)

            # every engine pre-observes the (single) DMA proc at its max tick
            dep_nop(nc.tensor, [t_rf[:], t_cE[:], t_ewR[:], t_nwT[:]])
            dep_nop(nc.scalar, [t_cbias[:], t_ebT[:], t_nbT[:]])
            dep_nop(nc.vector, [t_hT[:], t_maskF[:]])
            dep_nop(nc.gpsimd, [t_hT[:], t_maskF[:]])

            h00 = consts.tile([H, N], F32, tag="h00")
            nc.vector.tensor_copy(h00[:], t_hT[:])

            for rep in range(reps):
              if rep > 0:
                nc.vector.tensor_copy(t_hT[:], h00[:])

              # ---- stage 2: resident RBF table from host rf rows ----
              with tc.tile_pool(name="eps", bufs=2, space="PSUM") as eps:
                  for f in range(NFILL):
                      e = eps.tile([H, ECH], F32, tag="eps")
                      for s in range(ECH // MMF):
                          nc.tensor.matmul(
                              e[:, s * MMF:(s + 1) * MMF],
                              t_cE[:, f * H:(f + 1) * H],
                              t_rf[:, s * MMF:(s + 1) * MMF],
                              start=True, stop=True,
                          )
                      nc.scalar.activation(
                          rbfT[:, f * ECH:(f + 1) * ECH], e[:], AF.Exp,
                          bias=t_cbias[:],
                      )

              # ---- stage 3: message-passing layers ----
              with tc.tile_pool(name="lay", bufs=1) as lay, \
                   tc.tile_pool(name="gxp", bufs=2) as gxp, \
                   tc.tile_pool(name="gtp", bufs=2) as gtp, \
                   tc.tile_pool(name="ppp", bufs=2) as ppp, \
                   tc.tile_pool(name="tmp", bufs=2) as tmpp, \
                   tc.tile_pool(name="gpp", bufs=2, space="PSUM") as gpp:
                  hmr = lay.tile([H, N], BF16, tag="hmr0")
                  nc.vector.tensor_copy(hmr[:], t_hT[:])
                  for l in range(NL):
                      aggT = lay.tile([H, N], F32, tag=f"agg{l}")
                      for b in range(2 * G * (LOCF // BLK) // 2):
                          # block order: all g0 blocks, then all g1 blocks
                          g, bi = divmod(b, LOCF // BLK)
                          lf = bi * BLK
                          gx = gxp.tile([H, BLK], BF16, tag="gx")
                          for c in range(BLK // ECH):
                              gp = gpp.tile([H, ECH], F32, tag="gp")
                              for s in range(ECH // MMF):
                                  f0 = lf + c * ECH + s * MMF
                                  nc.tensor.matmul(
                                      gp[:, s * MMF:(s + 1) * MMF],
                                      t_ewR[64 * g:64 * g + BINS,
                                            l * H:(l + 1) * H],
                                      rbfT[64 * g:64 * g + BINS,
                                           f0:f0 + MMF],
                                      start=True, stop=True,
                                  )
                              nc.scalar.activation(
                                  gx[:, c * ECH:(c + 1) * ECH], gp[:],
                                  AF.Exp, bias=t_ebT[:, l:l + 1],
                              )
                          # softplus(x) = ln(exp(x) + 1); same ACT table set
                          gt = gtp.tile([H, BLK], BF16, tag="gt")
                          nc.scalar.activation(gt[:], gx[:], AF.Ln, bias=1.0)
                          # pp = gt * h_j (broadcast over i-rows)
                          pp = ppp.tile([H, BLK], BF16, tag="pp")
                          mul_eng.tensor_mul(
                              pp[:].rearrange("p (r c) -> p r c", c=N),
                              gt[:].rearrange("p (r c) -> p r c", c=N),
                              hmr[:, None, :].broadcast_to([H, IPB, N]),
                          )
                          # 2-step reduce over j: add halves, then reduce
                          tm = tmpp.tile([H, BLK // 2], BF16, tag="tm")
                          tmv = tm[:].rearrange("p (r c) -> p r c", c=N // 2)
                          ppv = pp[:].rearrange("p (r c) -> p r c", c=N)
                          mul_eng.tensor_add(
                              tmv, ppv[:, :, 0:N // 2], ppv[:, :, N // 2:N]
                          )
                          i0 = g * IPG + bi * IPB
                          nc.vector.reduce_sum(
                              out=aggT[:, i0:i0 + IPB], in_=tmv, axis=X.X,
                          )
                      # node update
                      dep_nop(nc.tensor, [aggT[:]])
                      zp = gpp.tile([H, ECH], F32, tag="gp")
                      nc.tensor.matmul(
                          zp[:, :N], t_nwT[:, l * H:(l + 1) * H], aggT[:],
                          start=True, stop=True,
                      )
                      sl = lay.tile([H, N], F32, tag=f"sil{l}")
                      nc.scalar.activation(
                          sl[:], zp[:, :N], AF.Silu, bias=t_nbT[:, l:l + 1],
                      )
                      h2 = lay.tile([H, N], F32, tag=f"h2_{l}")
                      nc.vector.tensor_add(h2[:], t_hT[:], sl[:])
                      nc.vector.tensor_mul(t_hT[:], h2[:], t_maskF[:])
                      if l + 1 < NL:
                          hmr = lay.tile([H, N], BF16, tag=f"hmr{l + 1}")
                          nc.vector.tensor_copy(hmr[:], t_hT[:])

                  sumh = lay.tile([H, 1], F32, tag="sumh")
                  nc.vector.reduce_sum(out=sumh[:], in_=t_hT[:], axis=X.X)
                  nc.gpsimd.dma_start(out=d_sumh[:], in_=sumh[:])

    return nc


def _get_nc(reps=1):
    key = f"nc{reps}"
    if key not in _CACHE:
        _CACHE[key] = _build_nc(reps)
    return _CACHE[key]


def _shared_inputs(edge_w, edge_b, node_w, node_b):
    centers = np.linspace(0.0, VMAX, BINS).astype(np.float64)
    # cE64: 16 per-fill selector blocks. Fill f uses rf rows 4f..4f+3:
    # row 4f+2g+0 = d^2 of group g, row 4f+2g+1 = d of group g.
    cE = np.zeros((64, NFILL * H), np.float32)
    for f in range(NFILL):
        for g in range(G):
            col0 = f * H + 64 * g
            cE[4 * f + 2 * g + 0, col0:col0 + BINS] = -GAMMA
            cE[4 * f + 2 * g + 1, col0:col0 + BINS] = 2.0 * GAMMA * centers
    cbias = np.zeros((H, 1), np.float32)
    ewR = np.zeros((H, NL * H), np.float32)
    for g in range(G):
        cbias[64 * g:64 * g + BINS, 0] = -GAMMA * centers * centers
        for l in range(NL):
            ewR[64 * g:64 * g + BINS, l * H:(l + 1) * H] = edge_w[l]
    ewR = ewR.astype(ml_dtypes.bfloat16)
    ebT = np.ascontiguousarray(edge_b.T).astype(np.float32)      # [H, NL]
    nwT = np.concatenate([node_w[l] for l in range(NL)], axis=1)
    nwT = np.ascontiguousarray(nwT).astype(np.float32)           # [H, NL*H]
    nbT = np.ascontiguousarray(node_b.T).astype(np.float32)      # [H, NL]
    return dict(cE64=cE, cbias=cbias, ewR=ewR, ebT=ebT, nwT=nwT, nbT=nbT)


def make_in_maps(atom_types, frac_coords, lattice, mask, emb_table,
                 edge_w, edge_b, node_w, node_b):
    shared = _shared_inputs(edge_w, edge_b, node_w, node_b)
    in_maps = []
    for b in range(B):
        cart = (frac_coords[b] @ lattice[b]).astype(np.float32)  # (N, 3)
        nsq = (cart * cart).sum(-1).astype(np.float32)
        d2 = nsq[:, None] + nsq[None, :] - 2.0 * (cart @ cart.T)
        d2 = np.maximum(d2, 0.0).astype(np.float32) + np.float32(1e-6)
        d = np.sqrt(d2)
        # rf64 [64, 2048]: fill f rows 4f+2g+{0,1} = (d^2, d) of group g,
        # i-rows [8f, 8f+8) of group g, row-major over (i, j).
        rf = np.empty((64, FILLP), np.float32)
        for f in range(NFILL):
            for g in range(G):
                i0 = g * IPG + f * IPF
                rf[4 * f + 2 * g + 0] = d2[i0:i0 + IPF].reshape(-1)
                rf[4 * f + 2 * g + 1] = d[i0:i0 + IPF].reshape(-1)
        types = np.where(mask[b], atom_types[b], 0).astype(np.int64)
        h0T = np.ascontiguousarray(emb_table[types].T).astype(np.float32)
        maskF = np.broadcast_to(
            mask[b].astype(np.float32)[None, :], (H, N)
        ).copy()
        in_maps.append(dict(rf64=rf, h0T=h0T, maskF=maskF, **shared))
    return in_maps


def kernel(**inputs):
    from concourse.bass_utils import run_bass_kernel_spmd

    atom_types = np.asarray(inputs["atom_types"])
    frac_coords = np.asarray(inputs["frac_coords"], np.float32)
    lattice = np.asarray(inputs["lattice"], np.float32)
    mask = np.asarray(inputs["mask"]).astype(bool)
    emb_table = np.asarray(inputs["emb_table"], np.float32)
    edge_w = np.asarray(inputs["edge_w"], np.float32)
    edge_b = np.asarray(inputs["edge_b"], np.float32)
    node_w = np.asarray(inputs["node_w"], np.float32)
    node_b = np.asarray(inputs["node_b"], np.float32)
    mu_w = np.asarray(inputs["mu_w"], np.float32)
    mu_b = np.asarray(inputs["mu_b"], np.float32)
    var_w = np.asarray(inputs["var_w"], np.float32)
    var_b = np.asarray(inputs["var_b"], np.float32)

    nc = _get_nc()
    in_maps = make_in_maps(atom_types, frac_coords, lattice, mask, emb_table,
                           edge_w, edge_b, node_w, node_b)
    res = run_bass_kernel_spmd(nc, in_maps, core_ids=list(range(B)))
    sum_h = np.stack([res.results[b]["sumh"][:, 0] for b in range(B)])
    n_valid = mask.sum(1).astype(np.float32)
    g = sum_h / (n_valid[:, None] + 1e-6)
    mu = (g @ mu_w + mu_b).astype(np.float32)
    log_var = (g @ var_w + var_b).astype(np.float32)
    return mu, log_var
